# revision 13
# baseline (speedup 1.0000x reference)
"""Graphormer attention (N=2048, D=512, H=8 heads of 64) on 8 NeuronCores.

Strategy (tensor-parallel over heads, one head per core):
  - Host slices Q/K/V/O projection weights per head, transposes x once.
  - The z-bin bias is folded in multiplicatively: W = exp(z_table[bin(z)])
    transposed to [key, query] layout, shipped as fp16.
  - On device (per core): fused Q^T/K^T projection (Q rows 0-63, K rows
    64-127), K/Q duplicated across both partition halves so S^T matmuls run
    as row-tiled pairs (two K=64 matmuls concurrently in the 128-row PE
    array).  exp on ScalarE -> fp16, P = exp(S) * W on VectorE,
    O'^T = sum_k V'[k,65] x P (65th V column = ones => row 64 of O' is the
    softmax denominator Z), then Y^T = Wo_h^T-tiles x O^T.
  - Host divides each head's partial Y by its Z, sums heads, adds biases.
"""

import numpy as np
import ml_dtypes
from contextlib import ExitStack

import concourse.bass as bass
import concourse.tile as tile
from concourse import bacc, mybir
from concourse import bass_utils

N = 2048
D = 512
H = 8
HD = 64
NUM_Z_BINS = 16
MAX_Z = 5.0
SCALE = HD ** -0.5
EXPA = 1024.0 / np.log(2.0)        # Schraudolph fp16 exponent scale
SIGMA = -60.0                      # Schraudolph bias correction (bit units)
DVE_TILES = (3, 8, 13)             # k-tiles whose exp runs on VectorE
NCORES = 8
QL = 1024          # query-chunk length (PSUM budget)
QC = N // QL       # 2 query chunks
KT = N // 128      # 16 key tiles
NP = KT // 2       # 8 key-tile pairs

FP32 = mybir.dt.float32
FP16 = mybir.dt.float16
BF16 = mybir.dt.bfloat16
BF16_NP = ml_dtypes.bfloat16
FP16_NP = np.float16

AF = mybir.ActivationFunctionType
OP = mybir.AluOpType

_PROGRAM_CACHE = {}


def _build_program():
    if "nc" in _PROGRAM_CACHE:
        return _PROGRAM_CACHE["nc"]

    nc = bacc.Bacc(
        "TRN2",
        target_bir_lowering=False,
        debug=False,
        enable_asserts=False,
        num_devices=NCORES,
    )

    xT = nc.dram_tensor("xT", [D, N], BF16, kind="ExternalInput").ap()
    wqk = nc.dram_tensor("wqk", [D, 128], BF16, kind="ExternalInput").ap()
    wv = nc.dram_tensor("wv", [D, HD], BF16, kind="ExternalInput").ap()
    wo = nc.dram_tensor("wo", [HD, D], FP16, kind="ExternalInput").ap()
    bqk = nc.dram_tensor("bqk", [128], FP32, kind="ExternalInput").ap()
    wt = nc.dram_tensor("wt", [N, N], FP16, kind="ExternalInput").ap()

    ypT = nc.dram_tensor("ypT", [D, N], FP16, kind="ExternalOutput").ap()
    zrow = nc.dram_tensor("zrow", [N], FP16, kind="ExternalOutput").ap()

    with tile.TileContext(nc) as tc:
        with ExitStack() as ctx:
            _emit(ctx, tc, xT, wqk, wv, wo, bqk, wt, ypT, zrow)
    nc.compile()
    _PROGRAM_CACHE["nc"] = nc
    return nc


def _emit(ctx, tc, xT, wqk, wv, wo, bqk, wt, ypT, zrow):
    nc = tc.nc
    CH = D // 128  # 4 contraction chunks of the model dim

    singles = ctx.enter_context(tc.tile_pool(name="singles", bufs=1))
    # PSUM: 8 banks. ps slots are [128,1024]fp32 (2 banks, 3 slots = 6 banks)
    # shared by warmup/qk-proj/v-proj/S-tiles/Y; ps_o (2 banks) holds O'.
    ps = ctx.enter_context(tc.tile_pool(name="ps", bufs=3, space="PSUM"))
    ps_o = ctx.enter_context(tc.tile_pool(name="ps_o", bufs=1, space="PSUM"))
    wpool = ctx.enter_context(tc.tile_pool(name="wpool", bufs=4))
    epool = ctx.enter_context(tc.tile_pool(name="epool", bufs=4))
    ppool = ctx.enter_context(tc.tile_pool(name="ppool", bufs=8))
    ypool = ctx.enter_context(tc.tile_pool(name="ypool", bufs=3))

    # ---- constants / inputs ---------------------------------------------
    # Each DMA instruction costs ~600ns of issue time on its HWDGE queue, so
    # transfers are batched (one per xT column chunk, one per weight) and
    # split across the two HWDGE queues (sync + scalar, idle at startup).
    # bqk goes absolutely first: the qT/kT evacuations -- and therefore the
    # whole S->exp pipeline -- wait on it.
    bqk_sb = singles.tile([128, 1], FP32)
    nc.sync.dma_start(out=bqk_sb, in_=bqk.rearrange("(n a) -> n a", a=1))
    xrow = xT.rearrange("(c p) n -> c p n", p=128)
    xc = []
    for c in range(CH):
        t_ = singles.tile([128, N], BF16, tag=f"xc{c}")
        xc.append(t_)
    for c in range(CH):
        nc.sync.dma_start(out=xc[c][:, 0:QL], in_=xrow[c, :, 0:QL])
    for c in range(CH):
        nc.sync.dma_start(out=xc[c][:, QL:N], in_=xrow[c, :, QL:N])

    # dummy exp to pull the ACT table load off the critical path; the
    # remaining input DMAs ride the scalar engine's HWDGE queue, which is
    # otherwise idle until the first real exp.
    dummy = singles.tile([128, 2], FP32)
    nc.vector.memset(dummy, 0.0)
    nc.scalar.activation(dummy[:, 1:2], dummy[:, 0:1], AF.Exp)

    wqk_sb = singles.tile([128, CH * 128], BF16)
    for c in range(CH):
        nc.scalar.dma_start(out=wqk_sb[:, c * 128:(c + 1) * 128],
                            in_=wqk.rearrange("(c p) m -> c p m", p=128)[c])
    wv_sb = singles.tile([128, CH * HD], BF16)
    for c in range(CH):
        nc.scalar.dma_start(out=wv_sb[:, c * HD:(c + 1) * HD],
                            in_=wv.rearrange("(c p) m -> c p m", p=128)[c])
    wo_sb = singles.tile([HD, D], FP16)
    nc.scalar.dma_start(out=wo_sb, in_=wo)

    # PE warm-up during the input DMA (HAM clock-gate release).
    scratch = singles.tile([128, 256], BF16)
    nc.vector.memset(scratch, 0.0)
    wu = ps.tile([128, 512], FP32, tag="big")
    for _ in range(4):
        nc.tensor.matmul(wu[:, 0:256], lhsT=scratch[:, 0:128], rhs=scratch,
                         start=True, stop=True)

    # ---- fused Q^T/K^T projection ---------------------------------------
    # pt rows 0-63 = Q^T, rows 64-127 = K^T for 1024 tokens per half.
    # Evacuation builds:
    #   qT_sb [128, N]: rows 0-63 = scaled Q^T, rows 64-127 = copy of it
    #   kT_sb [128, 8*128]: pair p cols, rows 0-63 = K^T tile 2p,
    #                       rows 64-127 = K^T tile 2p+1
    # so S^T runs as two concurrent row-tiled K=64 matmuls per pair.
    qT_sb = singles.tile([128, N], BF16)
    kT_sb = singles.tile([128, NP * 128], BF16)
    v_sb = singles.tile([128, KT * (HD + 1)], FP16)
    nc.vector.memset(v_sb, 1.0)
    oT_sb = singles.tile([HD + 1, N], FP16)

    def emit_qk_proj(h):
        pt = ps.tile([128, 1024], FP32, tag="big")
        for n in range(2):
            for c in range(CH):
                nc.tensor.matmul(
                    pt[:, n * 512:(n + 1) * 512],
                    lhsT=wqk_sb[:, c * 128:(c + 1) * 128],
                    rhs=xc[c][:, h * 1024 + n * 512:h * 1024 + (n + 1) * 512],
                    start=(c == 0),
                    stop=(c == CH - 1),
                )
        return pt

    def emit_qk_evac(h, pt, split):
        # kT: even tiles -> rows 0-63, odd tiles -> rows 64-127
        ptk = pt[64:128, :].rearrange("p (a b n) -> p a b n", b=2, n=128)
        kdst = kT_sb[:, h * 512:(h + 1) * 512].rearrange(
            "p (a n) -> p a n", n=128)
        qdst = qT_sb[:, h * 1024:(h + 1) * 1024]
        nsplit = 2 if split else 1
        w_ = 1024 // nsplit
        for s in range(nsplit):
            cs = slice(s * w_, (s + 1) * w_)
            nc.vector.tensor_scalar(qdst[0:64, cs], pt[0:64, cs],
                                    bqk_sb[0:HD, :], SCALE * EXPA, OP.add, OP.mult)
            a0, a1 = s * (4 // nsplit), (s + 1) * (4 // nsplit)
            nc.vector.tensor_scalar(kdst[0:64, a0:a1], ptk[:, a0:a1, 0],
                                    bqk_sb[HD:128, :], None, OP.add)
            nc.vector.tensor_scalar(kdst[64:128, a0:a1], ptk[:, a0:a1, 1],
                                    bqk_sb[HD:128, :], None, OP.add)
            nc.vector.tensor_copy(qdst[64:128, cs], qdst[0:64, cs])

    def emit_vgroup(g):
        # V' for key tiles 4g..4g+3 (col 64 of each 65-block is ones).
        vg = ps.tile([128, 256], FP32, tag="big")
        for ml in range(4):
            m = g * 4 + ml
            for c in range(CH):
                nc.tensor.matmul(
                    vg[:, ml * HD:(ml + 1) * HD],
                    lhsT=xc[c][:, m * 128:(m + 1) * 128],
                    rhs=wv_sb[:, c * HD:(c + 1) * HD],
                    start=(c == 0),
                    stop=(c == CH - 1),
                )
        dst = v_sb[:, g * 4 * (HD + 1):(g + 1) * 4 * (HD + 1)].rearrange(
            "p (t c) -> p t c", c=HD + 1)[:, :, 0:HD]
        nc.vector.tensor_copy(dst, vg.rearrange("p (t c) -> p t c", c=HD))

    # ---- main loop helpers ----------------------------------------------
    def emit_spair(qc, p, pending):
        # two concurrent K=64 matmuls: tile 2p in PE rows 0-63,
        # tile 2p+1 in rows 64-127.
        st_e = ps.tile([128, QL], FP32, tag="big")
        st_o = ps.tile([128, QL], FP32, tag="big")
        w_pair = wpool.tile([128, 2 * QL], FP16, tag="w")
        w_e = w_pair[:, 0:QL]
        w_o = w_pair[:, QL:2 * QL]
        t = 2 * p
        nc.sync.dma_start(out=w_e,
                          in_=wt[t * 128:(t + 1) * 128, qc * QL:(qc + 1) * QL])
        nc.sync.dma_start(
            out=w_o,
            in_=wt[(t + 1) * 128:(t + 2) * 128, qc * QL:(qc + 1) * QL])
        kcol = slice(p * 128, (p + 1) * 128)
        for n in range(QL // 512):
            qs = slice(qc * QL + n * 512, qc * QL + (n + 1) * 512)
            nc.tensor.matmul(st_e[:, n * 512:(n + 1) * 512],
                             lhsT=kT_sb[0:64, kcol], rhs=qT_sb[0:64, qs],
                             start=True, stop=True)
        for n in range(QL // 512):
            qs = slice(qc * QL + n * 512, qc * QL + (n + 1) * 512)
            nc.tensor.matmul(st_o[:, n * 512:(n + 1) * 512],
                             lhsT=kT_sb[64:128, kcol], rhs=qT_sb[64:128, qs],
                             start=True, stop=True)
        pending[p] = (st_e, st_o, w_e, w_o)

    def emit_tile(ot, t, st, w_tile):
        p_tile = ppool.tile([128, QL], FP16, tag="p")
        if t in DVE_TILES:
            # Schraudolph: fp16 bits of exp(s+b) ~= int16(EXPA*s + w'') with
            # w'' = EXPA*b + 15360 + sigma (shipped in wt); one VectorE op.
            nc.vector.tensor_tensor(p_tile.bitcast(mybir.dt.int16), st,
                                    w_tile, OP.add)
        else:
            e_tile = epool.tile([128, QL], FP16, tag="e")
            nc.scalar.activation(e_tile, st, AF.Exp, scale=1.0 / EXPA)
            nc.vector.tensor_mul(p_tile, e_tile, w_tile)
        for n in range(QL // 512):
            nc.tensor.matmul(
                ot[:, n * 512:(n + 1) * 512],
                lhsT=v_sb[:, t * (HD + 1):(t + 1) * (HD + 1)],
                rhs=p_tile[:, n * 512:(n + 1) * 512],
                start=(t == 0),
                stop=(t == KT - 1),
            )

    def emit_y(qc, ms, evac_engines):
        for i, m in enumerate(ms):
            yt = ps.tile([128, QL], FP32, tag="big")
            for n in range(QL // 512):
                nc.tensor.matmul(
                    yt[:, n * 512:(n + 1) * 512],
                    lhsT=wo_sb[:, m * 128:(m + 1) * 128],
                    rhs=oT_sb[0:HD, qc * QL + n * 512: qc * QL + (n + 1) * 512],
                    start=True,
                    stop=True,
                )
            y_sb = ypool.tile([128, QL], FP16, tag="ysb")
            if evac_engines[i] == "v":
                nc.vector.tensor_copy(y_sb, yt)
            else:
                nc.scalar.copy(y_sb, yt)
            nc.sync.dma_start(
                out=ypT[m * 128:(m + 1) * 128, qc * QL:(qc + 1) * QL],
                in_=y_sb,
            )

    # ---- emission schedule ----------------------------------------------
    pt0 = emit_qk_proj(0)
    emit_qk_evac(0, pt0, split=True)
    emit_vgroup(0)

    pend0 = {}
    emit_spair(0, 0, pend0)
    emit_vgroup(1)
    pt1 = emit_qk_proj(1)
    emit_qk_evac(1, pt1, split=False)
    emit_spair(0, 1, pend0)
    emit_vgroup(2)

    ot0 = ps_o.tile([HD + 1, QL], tag="ot", dtype=FP32)
    for p in range(NP):
        if p + 2 < NP:
            emit_spair(0, p + 2, pend0)
        if p == 0:
            emit_vgroup(3)
        st_e, st_o, w_e, w_o = pend0.pop(p)
        emit_tile(ot0, 2 * p, st_e, w_e)
        emit_tile(ot0, 2 * p + 1, st_o, w_o)
    nc.vector.tensor_copy(oT_sb[:, 0:QL], ot0)

    pend1 = {}
    emit_spair(1, 0, pend1)
    emit_spair(1, 1, pend1)
    ot1 = ps_o.tile([HD + 1, QL], tag="ot", dtype=FP32)
    for p in range(NP):
        if p + 2 < NP:
            emit_spair(1, p + 2, pend1)
        # qc0's output projection rides in qc1's PE/DVE slack
        if p in (2, 3, 4, 5):
            emit_y(0, [p - 2], ["v"])
        if p == 5:
            nc.sync.dma_start(
                out=zrow.rearrange("(a n) -> a n", a=1)[:, 0:QL],
                in_=oT_sb[HD:HD + 1, 0:QL])
        st_e, st_o, w_e, w_o = pend1.pop(p)
        emit_tile(ot1, 2 * p, st_e, w_e)
        emit_tile(ot1, 2 * p + 1, st_o, w_o)
    nc.scalar.copy(oT_sb[:, QL:2 * QL], ot1)

    emit_y(1, [0, 1], ["s", "v"])
    emit_y(1, [2, 3], ["s", "v"])
    nc.sync.dma_start(out=zrow.rearrange("(a n) -> a n", a=1)[:, QL:2 * QL],
                      in_=oT_sb[HD:HD + 1, QL:2 * QL])


def _install_ntff_hook():
    """Recreate the missing ``antenv.axon_hooks`` module so that
    run_bass_kernel_spmd(trace=True) can capture NTFF profiles via the
    libaxon_pjrt.so ctypes hook (see trn_agent_boot.trn_boot)."""
    import sys
    import types

    try:
        import antenv.axon_hooks  # noqa: F401
        return
    except ImportError:
        pass
    import antenv
    from trn_agent_boot.trn_boot import _ntff_profile_via_ctypes

    mod = types.ModuleType("antenv.axon_hooks")
    mod._hook = _ntff_profile_via_ctypes("/opt/axon/libaxon_pjrt.so")
    mod.set_axon_ntff_profile_hook = lambda h: setattr(mod, "_hook", h)
    mod.get_axon_ntff_profile_hook = lambda: mod._hook
    sys.modules["antenv.axon_hooks"] = mod
    antenv.axon_hooks = mod
    # keep profile artifacts local; the sandbox has no bucket access
    bass_utils.upload_artifacts = lambda tmpdir: tmpdir


def kernel(x, z_matrix, Wq, bq, Wk, bk, Wv, bv, Wo, bo, z_table, _trace=False):
    if _trace:
        _install_ntff_hook()
    x = np.ascontiguousarray(np.asarray(x, dtype=np.float32))
    z_matrix = np.asarray(z_matrix, dtype=np.float32)
    Wq = np.asarray(Wq, dtype=np.float32)
    Wk = np.asarray(Wk, dtype=np.float32)
    Wv = np.asarray(Wv, dtype=np.float32)
    Wo = np.asarray(Wo, dtype=np.float32)
    bq = np.asarray(bq, dtype=np.float32)
    bk = np.asarray(bk, dtype=np.float32)
    bv = np.asarray(bv, dtype=np.float32)
    bo = np.asarray(bo, dtype=np.float32)
    z_table = np.asarray(z_table, dtype=np.float32)

    nc = _build_program()

    xT = np.ascontiguousarray(x.T).astype(BF16_NP)
    binsT = np.clip(
        np.floor(z_matrix.T / MAX_Z * NUM_Z_BINS).astype(np.int32), 0, NUM_Z_BINS - 1
    )
    exp_tab = np.exp(z_table)  # [16, H] fp32

    in_maps = []
    for h in range(NCORES):
        sl = slice(h * HD, (h + 1) * HD)
        wt_h = exp_tab[:, h][binsT].astype(np.float32)  # [key, query] layout
        wadd = EXPA * z_table[:, h][binsT] + (15360.0 + SIGMA)
        for t_ in DVE_TILES:
            wt_h[t_ * 128:(t_ + 1) * 128, :] = wadd[t_ * 128:(t_ + 1) * 128, :]
        wt_h = wt_h.astype(FP16_NP)
        in_maps.append({
            "xT": xT,
            "wqk": np.ascontiguousarray(
                np.concatenate([Wq[:, sl], Wk[:, sl]], axis=1)).astype(BF16_NP),
            "wv": np.ascontiguousarray(Wv[:, sl]).astype(BF16_NP),
            "wo": np.ascontiguousarray(Wo[sl, :]).astype(FP16_NP),
            "bqk": np.concatenate([bq[sl], bk[sl]]),
            "wt": wt_h,
        })

    res = bass_utils.run_bass_kernel_spmd(
        nc, in_maps, core_ids=list(range(NCORES)), trace=_trace,
    )

    acc = np.zeros((D, N), dtype=np.float64)
    for h in range(NCORES):
        ypT_h = res.results[h]["ypT"].astype(np.float64)
        z_h = res.results[h]["zrow"].astype(np.float64)
        acc += ypT_h / z_h[None, :]
    out = acc.T + (bv @ Wo)[None, :] + bo[None, :]
    out_f32 = out.astype(np.float32)
    if _trace:
        return out_f32, res
    return out_f32


# revision 14
# speedup vs baseline: 1.0162x; 1.0162x over previous
"""Graphormer attention (N=2048, D=512, H=8 heads of 64) on 8 NeuronCores.

Strategy (tensor-parallel over heads, one head per core):
  - Host slices Q/K/V/O projection weights per head, transposes x once.
  - The z-bin bias is folded in multiplicatively: W = exp(z_table[bin(z)])
    transposed to [key, query] layout, shipped as fp16.
  - On device (per core): fused Q^T/K^T projection (Q rows 0-63, K rows
    64-127), K/Q duplicated across both partition halves so S^T matmuls run
    as row-tiled pairs (two K=64 matmuls concurrently in the 128-row PE
    array).  exp on ScalarE -> fp16, P = exp(S) * W on VectorE,
    O'^T = sum_k V'[k,65] x P (65th V column = ones => row 64 of O' is the
    softmax denominator Z), then Y^T = Wo_h^T-tiles x O^T.
  - Host divides each head's partial Y by its Z, sums heads, adds biases.
"""

import numpy as np
import ml_dtypes
from contextlib import ExitStack

import concourse.bass as bass
import concourse.tile as tile
from concourse import bacc, mybir
from concourse import bass_utils

N = 2048
D = 512
H = 8
HD = 64
NUM_Z_BINS = 16
MAX_Z = 5.0
SCALE = HD ** -0.5
NCORES = 8
QL = 1024          # query-chunk length (PSUM budget)
QC = N // QL       # 2 query chunks
KT = N // 128      # 16 key tiles
NP = KT // 2       # 8 key-tile pairs

FP32 = mybir.dt.float32
FP16 = mybir.dt.float16
BF16 = mybir.dt.bfloat16
BF16_NP = ml_dtypes.bfloat16
FP16_NP = np.float16

AF = mybir.ActivationFunctionType
OP = mybir.AluOpType

_PROGRAM_CACHE = {}


def _build_program():
    if "nc" in _PROGRAM_CACHE:
        return _PROGRAM_CACHE["nc"]

    nc = bacc.Bacc(
        "TRN2",
        target_bir_lowering=False,
        debug=False,
        enable_asserts=False,
        num_devices=NCORES,
    )

    xT = nc.dram_tensor("xT", [D, N], BF16, kind="ExternalInput").ap()
    wqk = nc.dram_tensor("wqk", [D, 128], BF16, kind="ExternalInput").ap()
    wv = nc.dram_tensor("wv", [D, HD], BF16, kind="ExternalInput").ap()
    wo = nc.dram_tensor("wo", [HD, D], FP16, kind="ExternalInput").ap()
    bqk = nc.dram_tensor("bqk", [128], FP32, kind="ExternalInput").ap()
    wt = nc.dram_tensor("wt", [N, N], FP16, kind="ExternalInput").ap()

    ypT = nc.dram_tensor("ypT", [D, N], FP16, kind="ExternalOutput").ap()
    zrow = nc.dram_tensor("zrow", [N], FP16, kind="ExternalOutput").ap()

    with tile.TileContext(nc) as tc:
        with ExitStack() as ctx:
            _emit(ctx, tc, xT, wqk, wv, wo, bqk, wt, ypT, zrow)
    nc.compile()
    _PROGRAM_CACHE["nc"] = nc
    return nc


def _emit(ctx, tc, xT, wqk, wv, wo, bqk, wt, ypT, zrow):
    nc = tc.nc
    CH = D // 128  # 4 contraction chunks of the model dim

    singles = ctx.enter_context(tc.tile_pool(name="singles", bufs=1))
    # PSUM: 8 banks. ps slots are [128,1024]fp32 (2 banks, 3 slots = 6 banks)
    # shared by warmup/qk-proj/v-proj/S-tiles/Y; ps_o (2 banks) holds O'.
    ps = ctx.enter_context(tc.tile_pool(name="ps", bufs=3, space="PSUM"))
    ps_o = ctx.enter_context(tc.tile_pool(name="ps_o", bufs=1, space="PSUM"))
    wpool = ctx.enter_context(tc.tile_pool(name="wpool", bufs=4))
    epool = ctx.enter_context(tc.tile_pool(name="epool", bufs=4))
    ppool = ctx.enter_context(tc.tile_pool(name="ppool", bufs=8))
    ypool = ctx.enter_context(tc.tile_pool(name="ypool", bufs=3))

    # ---- constants / inputs ---------------------------------------------
    # Each DMA instruction costs ~600ns of issue time on its HWDGE queue, so
    # transfers are batched (one per xT column chunk, one per weight) and
    # split across the two HWDGE queues (sync + scalar, idle at startup).
    # bqk goes absolutely first: the qT/kT evacuations -- and therefore the
    # whole S->exp pipeline -- wait on it.
    bqk_sb = singles.tile([128, 1], FP32)
    nc.sync.dma_start(out=bqk_sb, in_=bqk.rearrange("(n a) -> n a", a=1))
    xrow = xT.rearrange("(c p) n -> c p n", p=128)
    xc = []
    for c in range(CH):
        t_ = singles.tile([128, N], BF16, tag=f"xc{c}")
        xc.append(t_)
    for c in range(CH):
        nc.sync.dma_start(out=xc[c][:, 0:QL], in_=xrow[c, :, 0:QL])
    for c in range(CH):
        nc.sync.dma_start(out=xc[c][:, QL:N], in_=xrow[c, :, QL:N])

    # dummy exp to pull the ACT table load off the critical path; the
    # remaining input DMAs ride the scalar engine's HWDGE queue, which is
    # otherwise idle until the first real exp.
    dummy = singles.tile([128, 2], FP32)
    nc.vector.memset(dummy, 0.0)
    nc.scalar.activation(dummy[:, 1:2], dummy[:, 0:1], AF.Exp)

    wqk_sb = singles.tile([128, CH * 128], BF16)
    for c in range(CH):
        nc.scalar.dma_start(out=wqk_sb[:, c * 128:(c + 1) * 128],
                            in_=wqk.rearrange("(c p) m -> c p m", p=128)[c])
    wv_sb = singles.tile([128, CH * HD], BF16)
    for c in range(CH):
        nc.scalar.dma_start(out=wv_sb[:, c * HD:(c + 1) * HD],
                            in_=wv.rearrange("(c p) m -> c p m", p=128)[c])
    wo_sb = singles.tile([HD, D], FP16)
    nc.scalar.dma_start(out=wo_sb, in_=wo)

    # PE warm-up during the input DMA (HAM clock-gate release).
    scratch = singles.tile([128, 256], BF16)
    nc.vector.memset(scratch, 0.0)
    wu = ps.tile([128, 512], FP32, tag="big")
    for _ in range(4):
        nc.tensor.matmul(wu[:, 0:256], lhsT=scratch[:, 0:128], rhs=scratch,
                         start=True, stop=True)

    # ---- fused Q^T/K^T projection ---------------------------------------
    # pt rows 0-63 = Q^T, rows 64-127 = K^T for 1024 tokens per half.
    # Evacuation builds:
    #   qT_sb [128, N]: rows 0-63 = scaled Q^T, rows 64-127 = copy of it
    #   kT_sb [128, 8*128]: pair p cols, rows 0-63 = K^T tile 2p,
    #                       rows 64-127 = K^T tile 2p+1
    # so S^T runs as two concurrent row-tiled K=64 matmuls per pair.
    qT_sb = singles.tile([128, N], BF16)
    kT_sb = singles.tile([128, NP * 128], BF16)
    v_sb = singles.tile([128, KT * (HD + 1)], FP16)
    nc.vector.memset(v_sb, 1.0)
    oT_sb = singles.tile([HD + 1, N], FP16)

    def emit_qk_proj(h):
        pt = ps.tile([128, 1024], FP32, tag="big")
        for n in range(2):
            for c in range(CH):
                nc.tensor.matmul(
                    pt[:, n * 512:(n + 1) * 512],
                    lhsT=wqk_sb[:, c * 128:(c + 1) * 128],
                    rhs=xc[c][:, h * 1024 + n * 512:h * 1024 + (n + 1) * 512],
                    start=(c == 0),
                    stop=(c == CH - 1),
                )
        return pt

    def emit_qk_evac(h, pt, split):
        # kT: even tiles -> rows 0-63, odd tiles -> rows 64-127
        ptk = pt[64:128, :].rearrange("p (a b n) -> p a b n", b=2, n=128)
        kdst = kT_sb[:, h * 512:(h + 1) * 512].rearrange(
            "p (a n) -> p a n", n=128)
        qdst = qT_sb[:, h * 1024:(h + 1) * 1024]
        nsplit = 2 if split else 1
        w_ = 1024 // nsplit
        for s in range(nsplit):
            cs = slice(s * w_, (s + 1) * w_)
            nc.vector.tensor_scalar(qdst[0:64, cs], pt[0:64, cs],
                                    bqk_sb[0:HD, :], SCALE, OP.add, OP.mult)
            a0, a1 = s * (4 // nsplit), (s + 1) * (4 // nsplit)
            nc.vector.tensor_scalar(kdst[0:64, a0:a1], ptk[:, a0:a1, 0],
                                    bqk_sb[HD:128, :], None, OP.add)
            nc.vector.tensor_scalar(kdst[64:128, a0:a1], ptk[:, a0:a1, 1],
                                    bqk_sb[HD:128, :], None, OP.add)
            nc.vector.tensor_copy(qdst[64:128, cs], qdst[0:64, cs])

    def emit_vgroup(g):
        # V' for key tiles 4g..4g+3 (col 64 of each 65-block is ones).
        vg = ps.tile([128, 256], FP32, tag="big")
        for ml in range(4):
            m = g * 4 + ml
            for c in range(CH):
                nc.tensor.matmul(
                    vg[:, ml * HD:(ml + 1) * HD],
                    lhsT=xc[c][:, m * 128:(m + 1) * 128],
                    rhs=wv_sb[:, c * HD:(c + 1) * HD],
                    start=(c == 0),
                    stop=(c == CH - 1),
                )
        dst = v_sb[:, g * 4 * (HD + 1):(g + 1) * 4 * (HD + 1)].rearrange(
            "p (t c) -> p t c", c=HD + 1)[:, :, 0:HD]
        nc.vector.tensor_copy(dst, vg.rearrange("p (t c) -> p t c", c=HD))

    # ---- main loop helpers ----------------------------------------------
    def emit_spair(qc, p, pending):
        # two concurrent K=64 matmuls: tile 2p in PE rows 0-63,
        # tile 2p+1 in rows 64-127.
        st_e = ps.tile([128, QL], FP32, tag="big")
        st_o = ps.tile([128, QL], FP32, tag="big")
        w_pair = wpool.tile([128, 2 * QL], FP16, tag="w")
        w_e = w_pair[:, 0:QL]
        w_o = w_pair[:, QL:2 * QL]
        t = 2 * p
        nc.sync.dma_start(out=w_e,
                          in_=wt[t * 128:(t + 1) * 128, qc * QL:(qc + 1) * QL])
        nc.sync.dma_start(
            out=w_o,
            in_=wt[(t + 1) * 128:(t + 2) * 128, qc * QL:(qc + 1) * QL])
        kcol = slice(p * 128, (p + 1) * 128)
        for n in range(QL // 512):
            qs = slice(qc * QL + n * 512, qc * QL + (n + 1) * 512)
            nc.tensor.matmul(st_e[:, n * 512:(n + 1) * 512],
                             lhsT=kT_sb[0:64, kcol], rhs=qT_sb[0:64, qs],
                             start=True, stop=True)
        for n in range(QL // 512):
            qs = slice(qc * QL + n * 512, qc * QL + (n + 1) * 512)
            nc.tensor.matmul(st_o[:, n * 512:(n + 1) * 512],
                             lhsT=kT_sb[64:128, kcol], rhs=qT_sb[64:128, qs],
                             start=True, stop=True)
        pending[p] = (st_e, st_o, w_e, w_o)

    def emit_tile(ot, t, st, w_tile):
        e_tile = epool.tile([128, QL], FP16, tag="e")
        nc.scalar.activation(e_tile, st, AF.Exp)
        p_tile = ppool.tile([128, QL], FP16, tag="p")
        nc.vector.tensor_mul(p_tile, e_tile, w_tile)
        for n in range(QL // 512):
            nc.tensor.matmul(
                ot[:, n * 512:(n + 1) * 512],
                lhsT=v_sb[:, t * (HD + 1):(t + 1) * (HD + 1)],
                rhs=p_tile[:, n * 512:(n + 1) * 512],
                start=(t == 0),
                stop=(t == KT - 1),
            )

    def emit_y(qc, ms, evac_engines):
        for i, m in enumerate(ms):
            yt = ps.tile([128, QL], FP32, tag="big")
            for n in range(QL // 512):
                nc.tensor.matmul(
                    yt[:, n * 512:(n + 1) * 512],
                    lhsT=wo_sb[:, m * 128:(m + 1) * 128],
                    rhs=oT_sb[0:HD, qc * QL + n * 512: qc * QL + (n + 1) * 512],
                    start=True,
                    stop=True,
                )
            y_sb = ypool.tile([128, QL], FP16, tag="ysb")
            if evac_engines[i] == "v":
                nc.vector.tensor_copy(y_sb, yt)
            else:
                nc.scalar.copy(y_sb, yt)
            nc.sync.dma_start(
                out=ypT[m * 128:(m + 1) * 128, qc * QL:(qc + 1) * QL],
                in_=y_sb,
            )

    # ---- emission schedule ----------------------------------------------
    pt0 = emit_qk_proj(0)
    emit_qk_evac(0, pt0, split=True)
    emit_vgroup(0)

    pend0 = {}
    emit_spair(0, 0, pend0)
    emit_vgroup(1)
    pt1 = emit_qk_proj(1)
    emit_qk_evac(1, pt1, split=False)
    emit_spair(0, 1, pend0)
    emit_vgroup(2)

    ot0 = ps_o.tile([HD + 1, QL], tag="ot", dtype=FP32)
    for p in range(NP):
        if p + 2 < NP:
            emit_spair(0, p + 2, pend0)
        if p == 0:
            emit_vgroup(3)
        st_e, st_o, w_e, w_o = pend0.pop(p)
        emit_tile(ot0, 2 * p, st_e, w_e)
        emit_tile(ot0, 2 * p + 1, st_o, w_o)
    nc.vector.tensor_copy(oT_sb[:, 0:QL], ot0)

    pend1 = {}
    emit_spair(1, 0, pend1)
    emit_spair(1, 1, pend1)
    ot1 = ps_o.tile([HD + 1, QL], tag="ot", dtype=FP32)
    for p in range(NP):
        if p + 2 < NP:
            emit_spair(1, p + 2, pend1)
        # qc0's output projection rides in qc1's PE/DVE slack
        if p in (2, 3, 4, 5):
            emit_y(0, [p - 2], ["v"])
        if p == 5:
            nc.sync.dma_start(
                out=zrow.rearrange("(a n) -> a n", a=1)[:, 0:QL],
                in_=oT_sb[HD:HD + 1, 0:QL])
        st_e, st_o, w_e, w_o = pend1.pop(p)
        emit_tile(ot1, 2 * p, st_e, w_e)
        emit_tile(ot1, 2 * p + 1, st_o, w_o)
    nc.scalar.copy(oT_sb[:, QL:2 * QL], ot1)

    emit_y(1, [0, 1], ["s", "v"])
    emit_y(1, [2, 3], ["s", "v"])
    nc.sync.dma_start(out=zrow.rearrange("(a n) -> a n", a=1)[:, QL:2 * QL],
                      in_=oT_sb[HD:HD + 1, QL:2 * QL])


def _install_ntff_hook():
    """Recreate the missing ``antenv.axon_hooks`` module so that
    run_bass_kernel_spmd(trace=True) can capture NTFF profiles via the
    libaxon_pjrt.so ctypes hook (see trn_agent_boot.trn_boot)."""
    import sys
    import types

    try:
        import antenv.axon_hooks  # noqa: F401
        return
    except ImportError:
        pass
    import antenv
    from trn_agent_boot.trn_boot import _ntff_profile_via_ctypes

    mod = types.ModuleType("antenv.axon_hooks")
    mod._hook = _ntff_profile_via_ctypes("/opt/axon/libaxon_pjrt.so")
    mod.set_axon_ntff_profile_hook = lambda h: setattr(mod, "_hook", h)
    mod.get_axon_ntff_profile_hook = lambda: mod._hook
    sys.modules["antenv.axon_hooks"] = mod
    antenv.axon_hooks = mod
    # keep profile artifacts local; the sandbox has no bucket access
    bass_utils.upload_artifacts = lambda tmpdir: tmpdir


def kernel(x, z_matrix, Wq, bq, Wk, bk, Wv, bv, Wo, bo, z_table, _trace=False):
    if _trace:
        _install_ntff_hook()
    x = np.ascontiguousarray(np.asarray(x, dtype=np.float32))
    z_matrix = np.asarray(z_matrix, dtype=np.float32)
    Wq = np.asarray(Wq, dtype=np.float32)
    Wk = np.asarray(Wk, dtype=np.float32)
    Wv = np.asarray(Wv, dtype=np.float32)
    Wo = np.asarray(Wo, dtype=np.float32)
    bq = np.asarray(bq, dtype=np.float32)
    bk = np.asarray(bk, dtype=np.float32)
    bv = np.asarray(bv, dtype=np.float32)
    bo = np.asarray(bo, dtype=np.float32)
    z_table = np.asarray(z_table, dtype=np.float32)

    nc = _build_program()

    xT = np.ascontiguousarray(x.T).astype(BF16_NP)
    binsT = np.clip(
        np.floor(z_matrix.T / MAX_Z * NUM_Z_BINS).astype(np.int32), 0, NUM_Z_BINS - 1
    )
    exp_tab = np.exp(z_table)  # [16, H] fp32

    in_maps = []
    for h in range(NCORES):
        sl = slice(h * HD, (h + 1) * HD)
        wt_h = exp_tab[:, h][binsT].astype(FP16_NP)  # [key, query] layout
        in_maps.append({
            "xT": xT,
            "wqk": np.ascontiguousarray(
                np.concatenate([Wq[:, sl], Wk[:, sl]], axis=1)).astype(BF16_NP),
            "wv": np.ascontiguousarray(Wv[:, sl]).astype(BF16_NP),
            "wo": np.ascontiguousarray(Wo[sl, :]).astype(FP16_NP),
            "bqk": np.concatenate([bq[sl], bk[sl]]),
            "wt": wt_h,
        })

    res = bass_utils.run_bass_kernel_spmd(
        nc, in_maps, core_ids=list(range(NCORES)), trace=_trace,
    )

    acc = np.zeros((D, N), dtype=np.float64)
    for h in range(NCORES):
        ypT_h = res.results[h]["ypT"].astype(np.float64)
        z_h = res.results[h]["zrow"].astype(np.float64)
        acc += ypT_h / z_h[None, :]
    out = acc.T + (bv @ Wo)[None, :] + bo[None, :]
    out_f32 = out.astype(np.float32)
    if _trace:
        return out_f32, res
    return out_f32
